# revision 1
# baseline (speedup 1.0000x reference)
"""Trainium2 Bass kernel for ComplexMultiHeadAttentionV2.

Math reformulation: the reference's 5D phase tensor
    scores[b,h,q,k] = sum_d magq*magk*cos(delta*(ph_q - ph_k) + bias + (q-k)*freqs)
collapses via cos(a-b) = cos a cos b + sin a sin b with
    alpha[q,d] = delta*ph_q + bias + q*freqs   (query side)
    beta [k,d] = delta*ph_k + k*freqs          (key side)
into two rank-d_half matmuls:
    scores = (magq cos alpha) @ (magk cos beta).T + (magq sin alpha) @ (magk sin beta).T

Sharding: 8 cores = (batch 2) x (head-group 4); each core handles 1 batch and
2 heads. Host pre-transposes activations/weights so all device matmuls have
the contraction dim on partitions; host sums the 4 head-group partials per
batch at the end (W_o row-parallel).

Phases on device: atan2(im, r) = 2*atan(im / (mag + r)), division via the
custom-DVE approx reciprocal (guarded by +1e-30: den >= 0 always).
Softmax without max-subtraction (scores bounded ~55 for this problem size;
exp stays well inside fp32 range). Softmax denominator comes free from the
PV matmul via a ones-column appended to Vp; 1/l rows are PE-transposed into
partition-major form so the final per-head normalization fuses into the W_o
partial combine.
"""

import sys

for _p in ("/opt/trn_rl_repo",):
    if _p not in sys.path:
        sys.path.append(_p)

import numpy as np
from contextlib import ExitStack

import concourse.bass as bass
import concourse.tile as tile
from concourse import bacc, mybir
from concourse.bass_utils import run_bass_kernel_spmd

F32 = mybir.dt.float32
AF = mybir.ActivationFunctionType
ALU = mybir.AluOpType

N_CORES = 8
B, S, D = 2, 512, 512
H, D_K, D_HALF = 8, 64, 32
HPC = 2            # heads per core
SC = S // 128      # 4 seq chunks
JC = D // 128      # 4 contraction chunks
PI = float(np.pi)
MAGIC = 12582912.0                      # 1.5 * 2^23: fp32 round-to-nearest
TWO_PI_F = float(np.nextafter(np.float32(2 * np.pi), np.float32(0)))


def emit_body(nc, tc, ctx, aps, sb, ps):
    """Emit one full forward pass. aps: dict of DRAM APs. sb/ps: tile pools."""
    xq, xk, xv = aps["xq"], aps["xk"], aps["xv"]
    wq, wk, wv, wo = aps["wq"], aps["wk"], aps["wv"], aps["wo"]
    pf, dl2, out = aps["pf"], aps["dl2"], aps["out"]

    # ---- load inputs ------------------------------------------------------
    xq_sb = sb.tile([128, JC * 512], F32, tag="xq")
    xk_sb = sb.tile([128, JC * 512], F32, tag="xk")
    xv_sb = sb.tile([128, JC * 512], F32, tag="xv")
    for t, d in ((xq_sb, xq), (xk_sb, xk), (xv_sb, xv)):
        for jc in range(JC):
            nc.sync.dma_start(t[:, jc * 512:(jc + 1) * 512],
                              d[jc * 128:(jc + 1) * 128, :])
    wq_sb = sb.tile([128, JC * 128], F32, tag="wq")
    wk_sb = sb.tile([128, JC * 128], F32, tag="wk")
    wv_sb = sb.tile([128, JC * 128], F32, tag="wv")
    for t, d in ((wq_sb, wq), (wk_sb, wk), (wv_sb, wv)):
        for jc in range(JC):
            nc.sync.dma_start(t[:, jc * 128:(jc + 1) * 128],
                              d[jc * 128:(jc + 1) * 128, :])
    wo_sb = sb.tile([128, 512], F32, tag="wo")
    nc.sync.dma_start(wo_sb[:], wo[:])
    pf_sb = sb.tile([64, 1024], F32, tag="pf")
    nc.sync.dma_start(pf_sb[:], pf[:])
    dl2_sb = sb.tile([64, 1], F32, tag="dl2")
    nc.sync.dma_start(dl2_sb[:], dl2[:])

    # ---- projections (PE) -------------------------------------------------
    # QKT psum [64, 2048]: rows = (h0 d32, h1 d32); col quadrants
    # [Q-r 512 | Q-im 512 | K-r 512 | K-im 512]. r/im live on the FREE dim
    # (DVE tensor_tensor needs equal base partitions on its operands).
    qkt = ps.tile([64, 2048], F32, tag="qkt", bufs=1)
    for qd, (w_sb, x_sb, c0) in enumerate((
            (wq_sb, xq_sb, 0), (wq_sb, xq_sb, 64),
            (wk_sb, xk_sb, 0), (wk_sb, xk_sb, 64))):
        for jc in range(JC):
            nc.tensor.matmul(qkt[:, qd * 512:(qd + 1) * 512],
                             w_sb[:, jc * 128 + c0: jc * 128 + c0 + 64],
                             x_sb[:, jc * 512:(jc + 1) * 512],
                             start=(jc == 0), stop=(jc == JC - 1))
    # Vp psum [128, 512]: col-block sc holds [s-in-chunk, d'(h0 d64|h1 d64)]
    vp_ps = ps.tile([128, 512], F32, tag="ps512")
    for sc in range(SC):
        for jc in range(JC):
            nc.tensor.matmul(vp_ps[:, sc * 128:(sc + 1) * 128],
                             xv_sb[:, jc * 512 + sc * 128: jc * 512 + (sc + 1) * 128],
                             wv_sb[:, jc * 128:(jc + 1) * 128],
                             start=(jc == 0), stop=(jc == JC - 1))
    # Vp evac -> sbuf [128, 4*130]; per sc block: [h0 d64 | 1 | h1 d64 | 1].
    # Each head's PV lhsT is 65 contiguous cols [d64|1]; the ones column
    # lands the softmax denominator in out row 64 (partition-aligned for the
    # later psum reads).
    vp_sb = sb.tile([128, SC * 130], F32, tag="vp")
    for sc in range(SC):
        base = sc * 130
        nc.gpsimd.memset(vp_sb[:, base + 64: base + 65], 1.0)
        nc.gpsimd.memset(vp_sb[:, base + 129: base + 130], 1.0)
        nc.vector.tensor_copy(vp_sb[:, base: base + 64],
                              vp_ps[:, sc * 128: sc * 128 + 64])
        nc.vector.tensor_copy(vp_sb[:, base + 65: base + 129],
                              vp_ps[:, sc * 128 + 64:(sc + 1) * 128])

    # ---- mag/phase pipeline ----------------------------------------------
    # All tensors [64, 1024] free-packed: cols 0:512 = Q side, 512:1024 = K.
    # ACT table constraints: Arctan input must be in [-pi/2, pi/2], Sin in
    # [-pi, pi]. So: quarter-angle form theta = 4*atan(t4) with
    # t4 = im/(m1 + den) in [-1, 1] (den = mag + r, m1 = sqrt(den^2 + im^2)),
    # and angles tracked in TURNS with explicit round-to-nearest range
    # reduction before the Sin lookups.
    def v3(ap):
        return ap.rearrange("p (side s) -> p side s", side=2)[:]

    qkt4 = qkt.rearrange("p (side ri s) -> p side ri s", side=2, ri=2)
    r_v, im_v = qkt4[:, :, 0, :], qkt4[:, :, 1, :]      # [64, 2, 512]
    sq = sb.tile([64, 2048], F32, tag="sq")
    nc.scalar.square(sq[:], qkt[:])
    sq4 = sq.rearrange("p (side ri s) -> p side ri s", side=2, ri=2)
    mag2 = sb.tile([64, 1024], F32, tag="mag2")
    nc.vector.tensor_add(v3(mag2), sq4[:, :, 0, :], sq4[:, :, 1, :])
    mag = sb.tile([64, 1024], F32, tag="mag")
    nc.scalar.activation(mag[:], mag2[:], AF.Sqrt, bias=1e-9)
    den = sb.tile([64, 1024], F32, tag="den")
    nc.vector.tensor_add(v3(den), v3(mag), r_v)
    den2 = sb.tile([64, 1024], F32, tag="den2")
    nc.gpsimd.tensor_mul(den2[:], den[:], den[:])
    m1sq = sb.tile([64, 1024], F32, tag="m1sq")
    nc.gpsimd.tensor_add(v3(m1sq), v3(den2), sq4[:, :, 1, :])
    m1 = sb.tile([64, 1024], F32, tag="m1")
    nc.scalar.activation(m1[:], m1sq[:], AF.Sqrt)
    s_t = sb.tile([64, 1024], F32, tag="s_t")
    # s = (m1 + 1e-30) + den: eps guards recip when im = 0 and r = -mag
    nc.vector.scalar_tensor_tensor(s_t[:], m1[:], 1e-30, den[:],
                                   op0=ALU.add, op1=ALU.add)
    rs = sb.tile([64, 1024], F32, tag="rs")
    nc.vector.reciprocal_approx_fast(rs[:], s_t[:])
    t4 = sb.tile([64, 1024], F32, tag="t4")
    nc.vector.tensor_mul(v3(t4), im_v, v3(rs))
    atn = sb.tile([64, 1024], F32, tag="atn")
    nc.scalar.activation(atn[:], t4[:], AF.Arctan)
    # A_turns = (2*delta/pi)*atan + pf_turns  (theta*delta + pf, in turns)
    a_t = sb.tile([64, 1024], F32, tag="a_t")
    nc.vector.scalar_tensor_tensor(a_t[:], atn[:], dl2_sb[:, 0:1], pf_sb[:],
                                   op0=ALU.mult, op1=ALU.add)
    # k = round(A) via the 1.5*2^23 magic constant; f = A - k in [-.5, .5]
    k_t = sb.tile([64, 1024], F32, tag="k_t")
    nc.vector.tensor_scalar(k_t[:], a_t[:], MAGIC, MAGIC,
                            op0=ALU.add, op1=ALU.subtract)
    f_t = sb.tile([64, 1024], F32, tag="f_t")
    nc.vector.scalar_tensor_tensor(f_t[:], k_t[:], -1.0, a_t[:],
                                   op0=ALU.mult, op1=ALU.add)
    fs_t = sb.tile([64, 1024], F32, tag="fs_t")
    nc.vector.tensor_scalar(fs_t[:], f_t[:], 0.5, -0.5,
                            op0=ALU.min, op1=ALU.max)
    sin_a = sb.tile([64, 1024], F32, tag="sin_a")
    nc.scalar.activation(sin_a[:], fs_t[:], AF.Sin, scale=TWO_PI_F)
    # cos via sin(x + pi/2): +0.25 turns, wrapped back into [-.5, .5]
    g_t = sb.tile([64, 1024], F32, tag="g_t")
    nc.vector.add_range_wrap(g_t[:], f_t[:], 0.25, 0.5, 1.0)
    gs_t = sb.tile([64, 1024], F32, tag="gs_t")
    nc.vector.tensor_scalar(gs_t[:], g_t[:], 0.5, -0.5,
                            op0=ALU.min, op1=ALU.max)
    cos_a = sb.tile([64, 1024], F32, tag="cos_a")
    nc.scalar.activation(cos_a[:], gs_t[:], AF.Sin, scale=TWO_PI_F)
    ucos = sb.tile([64, 1024], F32, tag="ucos")
    nc.vector.tensor_mul(ucos[:], mag[:], cos_a[:])
    usin = sb.tile([64, 1024], F32, tag="usin")
    nc.gpsimd.tensor_mul(usin[:], mag[:], sin_a[:])

    # ---- scores (PE) + exp (ACT) -----------------------------------------
    # scoresT[k, q] per (head, k-chunk); exp into sbuf for PV rhs.
    exp_sb = [sb.tile([128, SC * 512], F32, tag=f"exp{h}", name=f"exp{h}")
              for h in range(HPC)]
    for h in range(HPC):
        r0 = 32 * h
        for kc in range(SC):
            sc_ps = ps.tile([128, 512], F32, tag="ps512")
            nc.tensor.matmul(sc_ps[:],
                             ucos[r0:r0 + 32, 512 + kc * 128: 512 + (kc + 1) * 128],
                             ucos[r0:r0 + 32, 0:512],
                             start=True, stop=False)
            nc.tensor.matmul(sc_ps[:],
                             usin[r0:r0 + 32, 512 + kc * 128: 512 + (kc + 1) * 128],
                             usin[r0:r0 + 32, 0:512],
                             start=False, stop=True)
            nc.scalar.activation(exp_sb[h][:, kc * 512:(kc + 1) * 512],
                                 sc_ps[:], AF.Exp)

    # ---- PV (PE): OutT_h (d-major) + l_h row (from the ones column) -------
    # h0: lhsT cols [1|d64] -> out row 0 = l, rows 1:65 = OutT
    # h1: lhsT cols [d64|1] -> out rows 0:64 = OutT, row 64 = l
    out_sb = sb.tile([128, 512], F32, tag="out_sb")
    rl_sb = [sb.tile([1, 512], F32, tag=f"rl{h}", name=f"rl{h}")
             for h in range(HPC)]
    for h in range(HPC):
        pv_ps = ps.tile([65, 512], F32, tag="ps512")
        for kc in range(SC):
            nc.tensor.matmul(pv_ps[:],
                             vp_sb[:, kc * 130 + 65 * h: kc * 130 + 65 * h + 65],
                             exp_sb[h][:, kc * 512:(kc + 1) * 512],
                             start=(kc == 0), stop=(kc == SC - 1))
        # custom-DVE ops give wrong results on HW when reading PSUM --
        # stage the l row through SBUF first
        l_sb = sb.tile([1, 512], F32, tag=f"l{h}", name=f"l{h}")
        nc.vector.tensor_copy(l_sb[:], pv_ps[64:65, :])
        nc.vector.reciprocal_approx_fast(rl_sb[h][:], l_sb[:])
        nc.vector.tensor_copy(out_sb[64 * h:64 * h + 64, :], pv_ps[0:64, :])

    # ---- 1/l transpose to partition-major (PE), W_o partials, combine -----
    one11 = nc.const_aps.aps[(F32, 1.0)][0:1, 0:1]
    for sc in range(SC):
        # 1/l rows -> partition-major via K=1 matmuls (out[m,0] = rl[m]*1)
        rlt_ps = ps.tile([128, 2], F32, tag="ps512")
        for h in range(HPC):
            nc.tensor.matmul(rlt_ps[:, h:h + 1],
                             rl_sb[h][0:1, sc * 128:(sc + 1) * 128], one11,
                             start=True, stop=True)
        rl_pm = sb.tile([128, 2], F32, tag="rl_pm")
        nc.vector.tensor_copy(rl_pm[:], rlt_ps[:])

        wo_ps = [ps.tile([128, 512], F32, tag="ps512", name=f"wo_ps{h}")
                 for h in range(HPC)]
        for h in range(HPC):
            nc.tensor.matmul(wo_ps[h][:],
                             out_sb[64 * h:64 * h + 64, sc * 128:(sc + 1) * 128],
                             wo_sb[64 * h:64 * h + 64, :],
                             start=True, stop=True)
        c1 = sb.tile([128, 512], F32, tag="c1")
        nc.vector.tensor_scalar_mul(c1[:], wo_ps[1][:], rl_pm[:, 1:2])
        fin = sb.tile([128, 512], F32, tag="fin")
        nc.vector.scalar_tensor_tensor(fin[:], wo_ps[0][:], rl_pm[:, 0:1], c1[:],
                                       op0=ALU.mult, op1=ALU.add)
        nc.sync.dma_start(out[sc * 128:(sc + 1) * 128, :], fin[:])


def build(reps=1):
    nc = bacc.Bacc("TRN2", target_bir_lowering=False, debug=False,
                   enable_asserts=False, num_devices=N_CORES)
    # Register const [128,1] SBUF tensors for the float biases used in
    # activation() calls (only 0.0/1.0 are pre-registered).
    for val in (1e-9, PI / 2):
        t = nc.alloc_sbuf_tensor(f"const-f32-{val}", [128, 1], F32)
        nc.gpsimd.memset(t.ap(), val)
        nc.const_aps.aps[(F32, val)] = t.ap()
    nc.all_engine_barrier()
    aps = {
        "xq": nc.dram_tensor("xq", [D, S], F32, kind="ExternalInput").ap(),
        "xk": nc.dram_tensor("xk", [D, S], F32, kind="ExternalInput").ap(),
        "xv": nc.dram_tensor("xv", [D, S], F32, kind="ExternalInput").ap(),
        "wq": nc.dram_tensor("wq", [D, 128], F32, kind="ExternalInput").ap(),
        "wk": nc.dram_tensor("wk", [D, 128], F32, kind="ExternalInput").ap(),
        "wv": nc.dram_tensor("wv", [D, 128], F32, kind="ExternalInput").ap(),
        "wo": nc.dram_tensor("wo", [128, D], F32, kind="ExternalInput").ap(),
        "pf": nc.dram_tensor("pf", [64, 1024], F32, kind="ExternalInput").ap(),
        "dl2": nc.dram_tensor("dl2", [64, 1], F32, kind="ExternalInput").ap(),
        "out": nc.dram_tensor("out", [S, D], F32, kind="ExternalOutput").ap(),
    }
    with tile.TileContext(nc) as tc:
        with ExitStack() as ctx:
            sb = ctx.enter_context(tc.tile_pool(name="sb", bufs=1))
            ps = ctx.enter_context(tc.tile_pool(name="ps", bufs=4, space="PSUM"))
            for _ in range(reps):
                emit_body(nc, tc, ctx, aps, sb, ps)
    nc.compile()
    return nc


def make_in_maps(q, k, v, W_q, W_k, W_v, W_o, delta_params, bias_params):
    """Host-side shard prep: per-core input dicts. Core c = 4*b + hg."""
    freqs = 10000.0 ** (-np.arange(D_HALF, dtype=np.float32) * 2.0 / D_K)
    pos = np.arange(S, dtype=np.float32)
    posfreq = pos[None, :] * freqs[:, None]          # [32, 512]
    in_maps = []
    for c in range(N_CORES):
        b, hg = divmod(c, 4)
        heads = [HPC * hg, HPC * hg + 1]
        # lhsT col layout for Q/K proj: [h0 r | h1 r | h0 im | h1 im]
        perm = []
        for ri in range(2):
            for h in heads:
                perm.extend(range(D_K * h + 32 * ri, D_K * h + 32 * ri + 32))
        pf = np.empty((64, 1024), np.float32)
        for i, h in enumerate(heads):
            rows = slice(32 * i, 32 * i + 32)
            pf[rows, 0:512] = (posfreq + bias_params[h][:, None]) / (2 * np.pi)
            pf[rows, 512:1024] = posfreq / (2 * np.pi)
        dl2 = np.empty((64, 1), np.float32)
        for i, h in enumerate(heads):
            dl2[32 * i:32 * i + 32, 0] = (2.0 / np.pi) * delta_params[h]
        hslc = slice(128 * hg, 128 * hg + 128)
        in_maps.append({
            "xq": np.ascontiguousarray(q[b].T),
            "xk": np.ascontiguousarray(k[b].T),
            "xv": np.ascontiguousarray(v[b].T),
            "wq": np.ascontiguousarray(W_q[perm, :].T),
            "wk": np.ascontiguousarray(W_k[perm, :].T),
            "wv": np.ascontiguousarray(W_v[hslc, :].T),
            "wo": np.ascontiguousarray(W_o[:, hslc].T),
            "pf": pf,
            "dl2": dl2,
        })
    return in_maps


_NC_CACHE = {}


def kernel(q, k, v, W_q, W_k, W_v, W_o, delta_params, bias_params):
    if "nc" not in _NC_CACHE:
        _NC_CACHE["nc"] = build(reps=1)
    nc = _NC_CACHE["nc"]
    in_maps = make_in_maps(q, k, v, W_q, W_k, W_v, W_o,
                           delta_params, bias_params)
    res = run_bass_kernel_spmd(nc, in_maps, core_ids=list(range(N_CORES)))
    outs = [res.results[c]["out"] for c in range(N_CORES)]
    final = np.empty((B, S, D), np.float32)
    for b in range(B):
        final[b] = outs[4 * b] + outs[4 * b + 1] + outs[4 * b + 2] + outs[4 * b + 3]
    return final



# revision 13
# speedup vs baseline: 366.5873x; 366.5873x over previous
"""Trainium2 Bass kernel for ComplexMultiHeadAttentionV2 (v2).

Math reformulation (same as v1): the reference's 5D phase tensor
    scores[b,h,q,k] = sum_d magq*magk*cos(delta*(ph_q - ph_k) + bias + (q-k)*freqs)
collapses via cos(a-b) = cos a cos b + sin a sin b with
    alpha[q,d] = delta*ph_q + bias + q*freqs   (query side)
    beta [k,d] = delta*ph_k + k*freqs          (key side)
into rank-d_half matmuls on U = [mag*cos; mag*sin].

v2 performance changes over v1:
  - Q/K projections in float32r (1 col/cycle vs 4 for fp32, full fp32 input
    precision feeding the phase math); value path (xv/wv/wo, exp, Vp, OutT)
    in bf16 (halves DMA + 1 col/cycle matmuls).
  - Phase pipeline packed [128, 512] (Q rows 0:64, K rows 64:128) instead of
    [64, 1024]: halves every DVE/ACT/Pool elementwise op.
  - cos|sin fused score matmuls: U tiles [64, 512] per (side, head) with
    cos on partitions 0:32, sin on 32:64 -> one contraction-64 matmul per
    (head, kchunk) instead of two.
  - Input DMAs spread across the three DMA rings (SP-HWDGE via nc.sync,
    ACT-HWDGE via nc.scalar, SWDGE via nc.gpsimd) instead of serializing on
    SP; weights are sent pre-packed in their SBUF image so each is one DMA.
  - ACT table churn minimized: Sqrt+Sqrt (sqrt set), Arctan+Sin+Sin (trig
    set), Exp x8 (exp set) -> 3 table loads per iteration.

Sharding: 8 cores = (batch 2) x (head-group 4); each core handles 1 batch and
2 heads; host sums the 4 head-group partials per batch (W_o row-parallel).
"""

import sys

for _p in ("/opt/trn_rl_repo",):
    if _p not in sys.path:
        sys.path.append(_p)

import numpy as np
import ml_dtypes
from contextlib import ExitStack

import concourse.bass as bass
import concourse.tile as tile
from concourse import bacc, mybir
from concourse.bass_utils import run_bass_kernel_spmd

F32 = mybir.dt.float32
F32R = mybir.dt.float32r
BF16 = mybir.dt.bfloat16
F16 = mybir.dt.float16
AF = mybir.ActivationFunctionType
ALU = mybir.AluOpType

N_CORES = 8
B, S, D = 2, 512, 512
H, D_K, D_HALF = 8, 64, 32
HPC = 2            # heads per core
SC = S // 128      # 4 seq chunks
JC = D // 128      # 4 contraction chunks
MAGIC = 12582912.0                      # 1.5 * 2^23: fp32 round-to-nearest
TWO_PI_F = float(np.nextafter(np.float32(2 * np.pi), np.float32(0)))


def emit_body(nc, tc, ctx, aps, sb, ps):
    """Emit one full forward pass. aps: dict of DRAM APs. sb/ps: tile pools."""
    xq, xk, xv = aps["xq"], aps["xk"], aps["xv"]
    wq, wk, wv, wo = aps["wq"], aps["wk"], aps["wv"], aps["wo"]
    pf, dl2, out = aps["pf"], aps["dl2"], aps["out"]

    one11 = nc.const_aps.aps[(F32, 1.0)][0:1, 0:1]

    # ---- input DMAs, spread across the three rings ------------------------
    # SP ring: xq chunks (+ output DMAs later). ACT ring: xk chunks.
    # SWDGE ring: weights + value path + phase tables.
    xq_sb = sb.tile([128, JC * 512], F32, tag="xq")
    for jc in range(JC):
        nc.sync.dma_start(xq_sb[:, jc * 512:(jc + 1) * 512],
                          xq[jc * 128:(jc + 1) * 128, :])
    xk_sb = sb.tile([128, JC * 512], F32, tag="xk")
    for jc in range(JC):
        nc.scalar.dma_start(xk_sb[:, jc * 512:(jc + 1) * 512],
                            xk[jc * 128:(jc + 1) * 128, :])
    wq_sb = sb.tile([128, 512], F32, tag="wq")
    nc.gpsimd.dma_start(wq_sb[:], wq[:])
    wk_sb = sb.tile([128, 512], F32, tag="wk")
    nc.gpsimd.dma_start(wk_sb[:], wk[:])
    wv_sb = sb.tile([128, 512], BF16, tag="wv")
    nc.gpsimd.dma_start(wv_sb[:], wv[:])
    xv_sb = sb.tile([128, JC * 512], BF16, tag="xv")
    xv3 = xv.rearrange("(a p) s -> p a s", p=128)
    xvs3 = xv_sb.rearrange("p (a s) -> p a s", s=512)
    nc.gpsimd.dma_start(xvs3[:, 0:2, :], xv3[:, 0:2, :])
    nc.gpsimd.dma_start(xvs3[:, 2:4, :], xv3[:, 2:4, :])
    wo_sb = sb.tile([128, 512], BF16, tag="wo")
    nc.gpsimd.dma_start(wo_sb[:], wo[:])
    pf_sb = sb.tile([128, 512], F32, tag="pf")
    nc.gpsimd.dma_start(pf_sb[:], pf[:])
    dl2_sb = sb.tile([128, 1], F32, tag="dl2")
    nc.gpsimd.dma_start(dl2_sb[:], dl2[:])

    # ---- projections (PE) -------------------------------------------------
    # Full fp32 (the phase -> exp amplification makes even tf32-precision
    # projections fail). One 128-wide-lhsT matmul per (side, jc) halves the
    # fp32 column count: psum rows = perm order [h0 r | h1 r | h0 im | h1 im].
    qktQ = ps.tile([128, 512], F32, tag="qktQ", bufs=1)
    qktK = ps.tile([128, 512], F32, tag="qktK", bufs=1)
    for dst, w_sb, x_sb in ((qktQ, wq_sb, xq_sb), (qktK, wk_sb, xk_sb)):
        for jc in range(JC):
            nc.tensor.matmul(dst[:],
                             w_sb[:, jc * 128:(jc + 1) * 128],
                             x_sb[:, jc * 512:(jc + 1) * 512],
                             start=(jc == 0), stop=(jc == JC - 1))
    # Vp psum [128, 512]: p = seq-in-chunk, col-block sc = [h0 d64 | h1 d64].
    vp_ps = ps.tile([128, 512], F32, tag="ps512")
    for sc in range(SC):
        for jc in range(JC):
            nc.tensor.matmul(vp_ps[:, sc * 128:(sc + 1) * 128],
                             xv_sb[:, jc * 512 + sc * 128: jc * 512 + (sc + 1) * 128],
                             wv_sb[:, jc * 128:(jc + 1) * 128],
                             start=(jc == 0), stop=(jc == JC - 1))

    # ---- mag/phase pipeline ([128, 512] packed: Q rows 0:64, K 64:128) ----
    # Quarter-angle: theta = 4*atan(t4), t4 = im/(m1 + den) in [-1, 1]
    # (den = mag + r, m1 = sqrt(den^2 + im^2)); angles in TURNS with magic
    # round-to-nearest range reduction before the Sin lookups. Ops that read
    # the projection psum run once per side (out partition offset 64 for K).
    # Evacuate both proj psums into one [128, 1024] SBUF tile (DVE takes Q,
    # ACT-Copy takes K, in parallel) so the whole chain runs as single
    # [128, *] ops. DVE can read the same SBUF region twice (it cannot on
    # PSUM), so sq needs no ACT Square.
    qkt_sb = sb.tile([128, 1024], F32, tag="qkt_sb")
    nc.vector.tensor_copy(qkt_sb[0:64, 0:512], qktQ[0:64, :])
    nc.vector.tensor_copy(qkt_sb[0:64, 512:1024], qktQ[64:128, :])
    nc.scalar.activation(qkt_sb[64:128, 0:512], qktK[0:64, :], AF.Copy)
    nc.scalar.activation(qkt_sb[64:128, 512:1024], qktK[64:128, :], AF.Copy)
    r_v, im_v = qkt_sb[:, 0:512], qkt_sb[:, 512:1024]
    sq = sb.tile([128, 1024], F32, tag="sq")
    nc.vector.tensor_mul(sq[:], qkt_sb[:], qkt_sb[:])
    sq_r, sq_im = sq[:, 0:512], sq[:, 512:1024]
    mag2 = sb.tile([128, 512], F32, tag="mag2")
    nc.vector.tensor_add(mag2[:], sq_r, sq_im)
    mag = sb.tile([128, 512], F32, tag="mag")
    nc.scalar.activation(mag[:], mag2[:], AF.Sqrt, bias=1e-9)
    den = sb.tile([128, 512], F32, tag="den")
    nc.vector.tensor_add(den[:], mag[:], r_v)
    den2 = sb.tile([128, 512], F32, tag="den2")
    nc.gpsimd.tensor_mul(den2[:], den[:], den[:])
    m1sq = sb.tile([128, 512], F32, tag="m1sq")
    nc.gpsimd.tensor_add(m1sq[:], den2[:], sq_im)
    m1 = sb.tile([128, 512], F32, tag="m1")
    nc.scalar.activation(m1[:], m1sq[:], AF.Sqrt)
    s_t = sb.tile([128, 512], F32, tag="s_t")
    # s = (m1 + 1e-30) + den: eps guards recip when im = 0 and r = -mag
    nc.vector.scalar_tensor_tensor(s_t[:], m1[:], 1e-30, den[:],
                                   op0=ALU.add, op1=ALU.add)
    rs = sb.tile([128, 512], F32, tag="rs")
    nc.vector.reciprocal_approx_fast(rs[:], s_t[:])
    t4 = sb.tile([128, 512], F32, tag="t4")
    nc.vector.tensor_mul(t4[:], im_v, rs[:])
    atn = sb.tile([128, 512], F32, tag="atn")
    nc.scalar.activation(atn[:], t4[:], AF.Arctan)
    # A_turns = (2*delta/pi)*atan + pf_turns
    a_t = sb.tile([128, 512], F32, tag="a_t")
    nc.vector.scalar_tensor_tensor(a_t[:], atn[:], dl2_sb[:, 0:1], pf_sb[:],
                                   op0=ALU.mult, op1=ALU.add)
    # k = round(A) via the 1.5*2^23 magic constant; f = A - k lands in
    # [-.5, .5] exactly (Sterbenz), so no clamp is needed before Sin.
    k_t = sb.tile([128, 512], F32, tag="k_t")
    nc.vector.tensor_scalar(k_t[:], a_t[:], MAGIC, MAGIC,
                            op0=ALU.add, op1=ALU.subtract)
    f_t = sb.tile([128, 512], F32, tag="f_t")
    nc.vector.scalar_tensor_tensor(f_t[:], k_t[:], -1.0, a_t[:],
                                   op0=ALU.mult, op1=ALU.add)
    sin_a = sb.tile([128, 512], F32, tag="sin_a")
    nc.scalar.activation(sin_a[:], f_t[:], AF.Sin, scale=TWO_PI_F)
    # cos via sin(x + pi/2): +0.25 turns, wrapped back into [-.5, .5]
    g_t = sb.tile([128, 512], F32, tag="g_t")
    nc.vector.add_range_wrap(g_t[:], f_t[:], 0.25, 0.5, 1.0)
    cos_a = sb.tile([128, 512], F32, tag="cos_a")
    nc.scalar.activation(cos_a[:], g_t[:], AF.Sin, scale=TWO_PI_F)

    # ---- Vp evac (f32r) + softmax-denominator ones columns ----------------
    # vp_sb [128, 4*130]; per sc block: [h0 d64 | 1 | h1 d64 | 1]. Each
    # head's PV lhsT is 65 contiguous cols; the ones column lands the
    # denominator in pv out row 64. memset can't write f32r, so the ones
    # come from an f32 scratch via a (rounding) DVE copy.
    ones8 = sb.tile([128, 8], F32, tag="ones8")
    nc.gpsimd.memset(ones8[:], 1.0)
    vp_sb = sb.tile([128, SC * 130], BF16, tag="vp_sb")
    vps3 = vp_sb.rearrange("p (a b) -> p a b", b=130)
    vpp3 = vp_ps.rearrange("p (a b) -> p a b", b=128)
    on3 = ones8.rearrange("p (a b) -> p a b", b=1)
    nc.vector.tensor_copy(vps3[:, :, 64:65], on3[:, 0:4, :])
    nc.vector.tensor_copy(vps3[:, :, 129:130], on3[:, 4:8, :])
    nc.vector.tensor_copy(vps3[:, :, 0:64], vpp3[:, :, 0:64])
    nc.vector.tensor_copy(vps3[:, :, 65:129], vpp3[:, :, 64:128])

    # ---- U tiles (bf16): per (side, head) [64, 512], cos rows 0:32,
    # sin rows 32:64. Partition-offset writes go to Pool (gpsimd shuffles
    # across partitions); the two aligned ones stay on DVE.
    uq = [sb.tile([64, 512], F16, tag=f"uq{h}", name=f"uq{h}") for h in range(HPC)]
    uk = [sb.tile([64, 512], F16, tag=f"uk{h}", name=f"uk{h}") for h in range(HPC)]
    nc.vector.tensor_mul(uq[0][0:32, :], mag[0:32, :], cos_a[0:32, :])
    nc.vector.tensor_mul(uq[0][32:64, :], mag[0:32, :], sin_a[0:32, :])
    nc.gpsimd.tensor_mul(uk[0][0:32, :], mag[64:96, :], cos_a[64:96, :])
    nc.gpsimd.tensor_mul(uk[0][32:64, :], mag[64:96, :], sin_a[64:96, :])
    nc.vector.tensor_mul(uq[1][0:32, :], mag[32:64, :], cos_a[32:64, :])
    nc.vector.tensor_mul(uq[1][32:64, :], mag[32:64, :], sin_a[32:64, :])
    nc.gpsimd.tensor_mul(uk[1][0:32, :], mag[96:128, :], cos_a[96:128, :])
    nc.gpsimd.tensor_mul(uk[1][32:64, :], mag[96:128, :], sin_a[96:128, :])

    # ---- scores (PE, one contraction-64 matmul per (h, kc)) + exp (ACT) ---
    exp_sb = [sb.tile([128, SC * 512], BF16, tag=f"exp{h}", name=f"exp{h}")
              for h in range(HPC)]
    for h in range(HPC):
        for kc in range(SC):
            sc_ps = ps.tile([128, 512], F32, tag="ps512")
            nc.tensor.matmul(sc_ps[:],
                             uk[h][:, kc * 128:(kc + 1) * 128],
                             uq[h][:, :],
                             start=True, stop=True)
            nc.scalar.activation(exp_sb[h][:, kc * 512:(kc + 1) * 512],
                                 sc_ps[:], AF.Exp)

    # ---- PV (PE): OutT_h rows 0:64 (d-major) + l row 64 -------------------
    out_sb = sb.tile([128, 512], BF16, tag="out_sb")
    rl_sb = [sb.tile([1, 512], F32, tag=f"rl{h}", name=f"rl{h}")
             for h in range(HPC)]
    for h in range(HPC):
        pv_ps = ps.tile([65, 512], F32, tag="ps512")
        for kc in range(SC):
            nc.tensor.matmul(pv_ps[:],
                             vp_sb[:, kc * 130 + 65 * h: kc * 130 + 65 * h + 65],
                             exp_sb[h][:, kc * 512:(kc + 1) * 512],
                             start=(kc == 0), stop=(kc == SC - 1))
        # custom-DVE ops give wrong results on HW when reading PSUM --
        # stage the l row through SBUF first
        l_sb = sb.tile([1, 512], F32, tag=f"l{h}", name=f"l{h}")
        nc.vector.tensor_copy(l_sb[:], pv_ps[64:65, :])
        nc.vector.reciprocal_approx_fast(rl_sb[h][:], l_sb[:])
        nc.vector.tensor_copy(out_sb[64 * h:64 * h + 64, :], pv_ps[0:64, :])

    # ---- 1/l transpose to partition-major (PE, one batched psum tile), ----
    # W_o partials, combine. rl_pm col 2*sc+h = 1/l for (chunk sc, head h).
    rlt_ps = ps.tile([128, 8], F32, tag="ps512")
    for sc in range(SC):
        for h in range(HPC):
            nc.tensor.matmul(rlt_ps[:, 2 * sc + h: 2 * sc + h + 1],
                             rl_sb[h][0:1, sc * 128:(sc + 1) * 128], one11,
                             start=True, stop=True)
    rl_pm = sb.tile([128, 8], F32, tag="rl_pm")
    nc.vector.tensor_copy(rl_pm[:], rlt_ps[:])

    for sc in range(SC):
        wo_ps = [ps.tile([128, 512], F32, tag="ps512", name=f"wo_ps{h}")
                 for h in range(HPC)]
        for h in range(HPC):
            nc.tensor.matmul(wo_ps[h][:],
                             out_sb[64 * h:64 * h + 64, sc * 128:(sc + 1) * 128],
                             wo_sb[64 * h:64 * h + 64, :],
                             start=True, stop=True)
        c1 = sb.tile([128, 512], F32, tag=f"c1_{sc}", name=f"c1_{sc}")
        nc.scalar.activation(c1[:], wo_ps[1][:], AF.Copy,
                             scale=rl_pm[:, 2 * sc + 1: 2 * sc + 2])
        fin = sb.tile([128, 512], BF16, tag=f"fin{sc}", name=f"fin{sc}")
        nc.vector.scalar_tensor_tensor(fin[:], wo_ps[0][:],
                                       rl_pm[:, 2 * sc: 2 * sc + 1], c1[:],
                                       op0=ALU.mult, op1=ALU.add)
        if sc % 2 == 0:
            nc.sync.dma_start(out[sc * 128:(sc + 1) * 128, :], fin[:])
        else:
            nc.scalar.dma_start(out[sc * 128:(sc + 1) * 128, :], fin[:])


def build(reps=1):
    nc = bacc.Bacc("TRN2", target_bir_lowering=False, debug=False,
                   enable_asserts=False, num_devices=N_CORES)
    # Const [128,1] SBUF tensor for the Sqrt bias (only 0.0/1.0 pre-registered).
    for val in (1e-9,):
        t = nc.alloc_sbuf_tensor(f"const-f32-{val}", [128, 1], F32)
        nc.gpsimd.memset(t.ap(), val)
        nc.const_aps.aps[(F32, val)] = t.ap()
    nc.all_engine_barrier()
    aps = {
        "xq": nc.dram_tensor("xq", [D, S], F32, kind="ExternalInput").ap(),
        "xk": nc.dram_tensor("xk", [D, S], F32, kind="ExternalInput").ap(),
        "xv": nc.dram_tensor("xv", [D, S], BF16, kind="ExternalInput").ap(),
        "wq": nc.dram_tensor("wq", [128, 512], F32, kind="ExternalInput").ap(),
        "wk": nc.dram_tensor("wk", [128, 512], F32, kind="ExternalInput").ap(),
        "wv": nc.dram_tensor("wv", [128, 512], BF16, kind="ExternalInput").ap(),
        "wo": nc.dram_tensor("wo", [128, 512], BF16, kind="ExternalInput").ap(),
        "pf": nc.dram_tensor("pf", [128, 512], F32, kind="ExternalInput").ap(),
        "dl2": nc.dram_tensor("dl2", [128, 1], F32, kind="ExternalInput").ap(),
        "out": nc.dram_tensor("out", [S, D], BF16, kind="ExternalOutput").ap(),
    }
    with tile.TileContext(nc) as tc:
        with ExitStack() as ctx:
            sb = ctx.enter_context(tc.tile_pool(name="sb", bufs=1))
            ps = ctx.enter_context(tc.tile_pool(name="ps", bufs=6, space="PSUM"))
            for _ in range(reps):
                emit_body(nc, tc, ctx, aps, sb, ps)
    nc.compile()
    return nc


def make_in_maps(q, k, v, W_q, W_k, W_v, W_o, delta_params, bias_params):
    """Host-side shard prep: per-core input dicts. Core c = 4*b + hg."""
    bf16 = ml_dtypes.bfloat16
    freqs = 10000.0 ** (-np.arange(D_HALF, dtype=np.float32) * 2.0 / D_K)
    pos = np.arange(S, dtype=np.float32)
    posfreq = (freqs[:, None] * pos[None, :]).astype(np.float32)  # [32, 512]

    def sbuf_img(w_t, dtype):
        # [512, m] (contraction-major) -> SBUF image [128, 4*m]
        m = w_t.shape[1]
        return np.ascontiguousarray(
            w_t.reshape(JC, 128, m).transpose(1, 0, 2).reshape(128, JC * m)
        ).astype(dtype)

    xqs = [np.ascontiguousarray(np.asarray(q[b]).T, dtype=np.float32) for b in range(B)]
    xks = [np.ascontiguousarray(np.asarray(k[b]).T, dtype=np.float32) for b in range(B)]
    xvs = [np.ascontiguousarray(np.asarray(v[b]).T).astype(bf16) for b in range(B)]

    per_hg = []
    for hg in range(4):
        heads = [HPC * hg, HPC * hg + 1]
        perm = []
        for ri in range(2):
            for h in heads:
                perm.extend(range(D_K * h + 32 * ri, D_K * h + 32 * ri + 32))
        hslc = slice(128 * hg, 128 * hg + 128)
        pf = np.empty((128, 512), np.float32)
        dl2 = np.empty((128, 1), np.float32)
        for i, h in enumerate(heads):
            qr = slice(32 * i, 32 * i + 32)
            kr = slice(64 + 32 * i, 64 + 32 * i + 32)
            pf[qr] = (posfreq + np.asarray(bias_params)[h][:, None]) / (2 * np.pi)
            pf[kr] = posfreq / (2 * np.pi)
            dl2[qr, 0] = (2.0 / np.pi) * np.asarray(delta_params)[h]
            dl2[kr, 0] = (2.0 / np.pi) * np.asarray(delta_params)[h]
        per_hg.append({
            "wq": sbuf_img(np.asarray(W_q)[perm, :].T, np.float32),
            "wk": sbuf_img(np.asarray(W_k)[perm, :].T, np.float32),
            "wv": sbuf_img(np.asarray(W_v)[hslc, :].T, bf16),
            "wo": np.ascontiguousarray(np.asarray(W_o)[:, hslc].T).astype(bf16),
            "pf": pf,
            "dl2": dl2,
        })

    in_maps = []
    for c in range(N_CORES):
        b, hg = divmod(c, 4)
        m = dict(per_hg[hg])
        m["xq"] = xqs[b]
        m["xk"] = xks[b]
        m["xv"] = xvs[b]
        in_maps.append(m)
    return in_maps


_NC_CACHE = {}


def kernel(q, k, v, W_q, W_k, W_v, W_o, delta_params, bias_params):
    if "nc" not in _NC_CACHE:
        _NC_CACHE["nc"] = build(reps=1)
    nc = _NC_CACHE["nc"]
    in_maps = make_in_maps(q, k, v, W_q, W_k, W_v, W_o,
                           delta_params, bias_params)
    res = run_bass_kernel_spmd(nc, in_maps, core_ids=list(range(N_CORES)))
    outs = [res.results[c]["out"].astype(np.float32) for c in range(N_CORES)]
    final = np.empty((B, S, D), np.float32)
    for b in range(B):
        final[b] = outs[4 * b] + outs[4 * b + 1] + outs[4 * b + 2] + outs[4 * b + 3]
    return final


# revision 14
# speedup vs baseline: 1058.8167x; 2.8883x over previous
"""Trainium2 Bass kernel for ComplexMultiHeadAttentionV2 (v2).

Math reformulation (same as v1): the reference's 5D phase tensor
    scores[b,h,q,k] = sum_d magq*magk*cos(delta*(ph_q - ph_k) + bias + (q-k)*freqs)
collapses via cos(a-b) = cos a cos b + sin a sin b with
    alpha[q,d] = delta*ph_q + bias + q*freqs   (query side)
    beta [k,d] = delta*ph_k + k*freqs          (key side)
into rank-d_half matmuls on U = [mag*cos; mag*sin].

v2 performance changes over v1:
  - Q/K projections in float32r (1 col/cycle vs 4 for fp32, full fp32 input
    precision feeding the phase math); value path (xv/wv/wo, exp, Vp, OutT)
    in bf16 (halves DMA + 1 col/cycle matmuls).
  - Phase pipeline packed [128, 512] (Q rows 0:64, K rows 64:128) instead of
    [64, 1024]: halves every DVE/ACT/Pool elementwise op.
  - cos|sin fused score matmuls: U tiles [64, 512] per (side, head) with
    cos on partitions 0:32, sin on 32:64 -> one contraction-64 matmul per
    (head, kchunk) instead of two.
  - Input DMAs spread across the three DMA rings (SP-HWDGE via nc.sync,
    ACT-HWDGE via nc.scalar, SWDGE via nc.gpsimd) instead of serializing on
    SP; weights are sent pre-packed in their SBUF image so each is one DMA.
  - ACT table churn minimized: Sqrt+Sqrt (sqrt set), Arctan+Sin+Sin (trig
    set), Exp x8 (exp set) -> 3 table loads per iteration.

Sharding: 8 cores = (batch 2) x (head-group 4); each core handles 1 batch and
2 heads; host sums the 4 head-group partials per batch (W_o row-parallel).
"""

import sys

for _p in ("/opt/trn_rl_repo",):
    if _p not in sys.path:
        sys.path.append(_p)

import numpy as np
import ml_dtypes
from contextlib import ExitStack

import concourse.bass as bass
import concourse.tile as tile
from concourse import bacc, mybir
from concourse.bass_utils import run_bass_kernel_spmd

F32 = mybir.dt.float32
F32R = mybir.dt.float32r
BF16 = mybir.dt.bfloat16
F16 = mybir.dt.float16
AF = mybir.ActivationFunctionType
ALU = mybir.AluOpType

N_CORES = 8
B, S, D = 2, 512, 512
H, D_K, D_HALF = 8, 64, 32
HPC = 2            # heads per core
SC = S // 128      # 4 seq chunks
JC = D // 128      # 4 contraction chunks
MAGIC = 12582912.0                      # 1.5 * 2^23: fp32 round-to-nearest
TWO_PI_F = float(np.nextafter(np.float32(2 * np.pi), np.float32(0)))


def emit_body(nc, tc, ctx, aps, sb, ps):
    """Emit one full forward pass. aps: dict of DRAM APs. sb/ps: tile pools."""
    xq, xk, xv = aps["xq"], aps["xk"], aps["xv"]
    wq, wk, wv, wo = aps["wq"], aps["wk"], aps["wv"], aps["wo"]
    pf, dl2, out = aps["pf"], aps["dl2"], aps["out"]

    one11 = nc.const_aps.aps[(F32, 1.0)][0:1, 0:1]

    # ---- input DMAs, spread across the three rings ------------------------
    # SP ring: xq chunks (+ output DMAs later). ACT ring: xk chunks.
    # SWDGE ring: weights + value path + phase tables.
    xq_sb = sb.tile([128, JC * 512], F32, tag="xq")
    for jc in range(JC):
        nc.sync.dma_start(xq_sb[:, jc * 512:(jc + 1) * 512],
                          xq[jc * 128:(jc + 1) * 128, :])
    xk_sb = sb.tile([128, JC * 512], F32, tag="xk")
    for jc in range(JC):
        nc.scalar.dma_start(xk_sb[:, jc * 512:(jc + 1) * 512],
                            xk[jc * 128:(jc + 1) * 128, :])
    wq_sb = sb.tile([128, 512], F32, tag="wq")
    nc.sync.dma_start(wq_sb[:], wq[:])
    wk_sb = sb.tile([128, 512], F32, tag="wk")
    nc.scalar.dma_start(wk_sb[:], wk[:])
    wv_sb = sb.tile([128, 512], BF16, tag="wv")
    nc.sync.dma_start(wv_sb[:], wv[:])
    xv_sb = sb.tile([128, JC * 512], BF16, tag="xv")
    xv3 = xv.rearrange("(a p) s -> p a s", p=128)
    xvs3 = xv_sb.rearrange("p (a s) -> p a s", s=512)
    nc.sync.dma_start(xvs3[:, 0:2, :], xv3[:, 0:2, :])
    nc.sync.dma_start(xvs3[:, 2:4, :], xv3[:, 2:4, :])
    wo_sb = sb.tile([128, 512], BF16, tag="wo")
    nc.scalar.dma_start(wo_sb[:], wo[:])
    pf_sb = sb.tile([128, 512], F32, tag="pf")
    nc.scalar.dma_start(pf_sb[:], pf[:])
    dl2_sb = sb.tile([128, 1], F32, tag="dl2")
    nc.scalar.dma_start(dl2_sb[:], dl2[:])

    # ---- projections (PE) -------------------------------------------------
    # Full fp32 (the phase -> exp amplification makes even tf32-precision
    # projections fail). One 128-wide-lhsT matmul per (side, jc) halves the
    # fp32 column count: psum rows = perm order [h0 r | h1 r | h0 im | h1 im].
    qktQ = ps.tile([128, 512], F32, tag="qktQ", bufs=1)
    qktK = ps.tile([128, 512], F32, tag="qktK", bufs=1)
    for dst, w_sb, x_sb in ((qktQ, wq_sb, xq_sb), (qktK, wk_sb, xk_sb)):
        for jc in range(JC):
            nc.tensor.matmul(dst[:],
                             w_sb[:, jc * 128:(jc + 1) * 128],
                             x_sb[:, jc * 512:(jc + 1) * 512],
                             start=(jc == 0), stop=(jc == JC - 1))
    # Vp psum [128, 512]: p = seq-in-chunk, col-block sc = [h0 d64 | h1 d64].
    vp_ps = ps.tile([128, 512], F32, tag="ps512")
    for sc in range(SC):
        for jc in range(JC):
            nc.tensor.matmul(vp_ps[:, sc * 128:(sc + 1) * 128],
                             xv_sb[:, jc * 512 + sc * 128: jc * 512 + (sc + 1) * 128],
                             wv_sb[:, jc * 128:(jc + 1) * 128],
                             start=(jc == 0), stop=(jc == JC - 1))

    # ---- mag/phase pipeline ([128, 512] packed: Q rows 0:64, K 64:128) ----
    # Quarter-angle: theta = 4*atan(t4), t4 = im/(m1 + den) in [-1, 1]
    # (den = mag + r, m1 = sqrt(den^2 + im^2)); angles in TURNS with magic
    # round-to-nearest range reduction before the Sin lookups. Ops that read
    # the projection psum run once per side (out partition offset 64 for K).
    # Evacuate both proj psums into one [128, 1024] SBUF tile (DVE takes Q,
    # ACT-Copy takes K, in parallel) so the whole chain runs as single
    # [128, *] ops. DVE can read the same SBUF region twice (it cannot on
    # PSUM), so sq needs no ACT Square.
    qkt_sb = sb.tile([128, 1024], F32, tag="qkt_sb")
    nc.vector.tensor_copy(qkt_sb[0:64, 0:512], qktQ[0:64, :])
    nc.vector.tensor_copy(qkt_sb[0:64, 512:1024], qktQ[64:128, :])
    nc.scalar.activation(qkt_sb[64:128, 0:512], qktK[0:64, :], AF.Copy)
    nc.scalar.activation(qkt_sb[64:128, 512:1024], qktK[64:128, :], AF.Copy)
    r_v, im_v = qkt_sb[:, 0:512], qkt_sb[:, 512:1024]
    sq = sb.tile([128, 1024], F32, tag="sq")
    nc.vector.tensor_mul(sq[:], qkt_sb[:], qkt_sb[:])
    sq_r, sq_im = sq[:, 0:512], sq[:, 512:1024]
    mag2 = sb.tile([128, 512], F32, tag="mag2")
    nc.vector.tensor_add(mag2[:], sq_r, sq_im)
    mag = sb.tile([128, 512], F32, tag="mag")
    nc.scalar.activation(mag[:], mag2[:], AF.Sqrt, bias=1e-9)
    den = sb.tile([128, 512], F32, tag="den")
    nc.vector.tensor_add(den[:], mag[:], r_v)
    den2 = sb.tile([128, 512], F32, tag="den2")
    nc.gpsimd.tensor_mul(den2[:], den[:], den[:])
    m1sq = sb.tile([128, 512], F32, tag="m1sq")
    nc.gpsimd.tensor_add(m1sq[:], den2[:], sq_im)
    m1 = sb.tile([128, 512], F32, tag="m1")
    nc.scalar.activation(m1[:], m1sq[:], AF.Sqrt)
    s_t = sb.tile([128, 512], F32, tag="s_t")
    # s = (m1 + 1e-30) + den: eps guards recip when im = 0 and r = -mag
    nc.vector.scalar_tensor_tensor(s_t[:], m1[:], 1e-30, den[:],
                                   op0=ALU.add, op1=ALU.add)
    rs = sb.tile([128, 512], F32, tag="rs")
    nc.vector.reciprocal_approx_fast(rs[:], s_t[:])
    t4 = sb.tile([128, 512], F32, tag="t4")
    nc.vector.tensor_mul(t4[:], im_v, rs[:])
    atn = sb.tile([128, 512], F32, tag="atn")
    nc.scalar.activation(atn[:], t4[:], AF.Arctan)
    # A_turns = (2*delta/pi)*atan + pf_turns
    a_t = sb.tile([128, 512], F32, tag="a_t")
    nc.vector.scalar_tensor_tensor(a_t[:], atn[:], dl2_sb[:, 0:1], pf_sb[:],
                                   op0=ALU.mult, op1=ALU.add)
    # k = round(A) via the 1.5*2^23 magic constant; f = A - k lands in
    # [-.5, .5] exactly (Sterbenz), so no clamp is needed before Sin.
    k_t = sb.tile([128, 512], F32, tag="k_t")
    nc.vector.tensor_scalar(k_t[:], a_t[:], MAGIC, MAGIC,
                            op0=ALU.add, op1=ALU.subtract)
    f_t = sb.tile([128, 512], F32, tag="f_t")
    nc.vector.scalar_tensor_tensor(f_t[:], k_t[:], -1.0, a_t[:],
                                   op0=ALU.mult, op1=ALU.add)
    sin_a = sb.tile([128, 512], F32, tag="sin_a")
    nc.scalar.activation(sin_a[:], f_t[:], AF.Sin, scale=TWO_PI_F)
    # cos via sin(x + pi/2): +0.25 turns, wrapped back into [-.5, .5]
    g_t = sb.tile([128, 512], F32, tag="g_t")
    nc.vector.add_range_wrap(g_t[:], f_t[:], 0.25, 0.5, 1.0)
    cos_a = sb.tile([128, 512], F32, tag="cos_a")
    nc.scalar.activation(cos_a[:], g_t[:], AF.Sin, scale=TWO_PI_F)

    # ---- Vp evac (f32r) + softmax-denominator ones columns ----------------
    # vp_sb [128, 4*130]; per sc block: [h0 d64 | 1 | h1 d64 | 1]. Each
    # head's PV lhsT is 65 contiguous cols; the ones column lands the
    # denominator in pv out row 64. memset can't write f32r, so the ones
    # come from an f32 scratch via a (rounding) DVE copy.
    ones8 = sb.tile([128, 8], F32, tag="ones8")
    nc.gpsimd.memset(ones8[:], 1.0)
    vp_sb = sb.tile([128, SC * 130], BF16, tag="vp_sb")
    vps3 = vp_sb.rearrange("p (a b) -> p a b", b=130)
    vpp3 = vp_ps.rearrange("p (a b) -> p a b", b=128)
    on3 = ones8.rearrange("p (a b) -> p a b", b=1)
    nc.vector.tensor_copy(vps3[:, :, 64:65], on3[:, 0:4, :])
    nc.vector.tensor_copy(vps3[:, :, 129:130], on3[:, 4:8, :])
    nc.vector.tensor_copy(vps3[:, :, 0:64], vpp3[:, :, 0:64])
    nc.vector.tensor_copy(vps3[:, :, 65:129], vpp3[:, :, 64:128])

    # ---- U tiles (bf16): per (side, head) [64, 512], cos rows 0:32,
    # sin rows 32:64. Partition-offset writes go to Pool (gpsimd shuffles
    # across partitions); the two aligned ones stay on DVE.
    uq = [sb.tile([64, 512], F16, tag=f"uq{h}", name=f"uq{h}") for h in range(HPC)]
    uk = [sb.tile([64, 512], F16, tag=f"uk{h}", name=f"uk{h}") for h in range(HPC)]
    nc.vector.tensor_mul(uq[0][0:32, :], mag[0:32, :], cos_a[0:32, :])
    nc.vector.tensor_mul(uq[0][32:64, :], mag[0:32, :], sin_a[0:32, :])
    nc.gpsimd.tensor_mul(uk[0][0:32, :], mag[64:96, :], cos_a[64:96, :])
    nc.gpsimd.tensor_mul(uk[0][32:64, :], mag[64:96, :], sin_a[64:96, :])
    nc.vector.tensor_mul(uq[1][0:32, :], mag[32:64, :], cos_a[32:64, :])
    nc.vector.tensor_mul(uq[1][32:64, :], mag[32:64, :], sin_a[32:64, :])
    nc.gpsimd.tensor_mul(uk[1][0:32, :], mag[96:128, :], cos_a[96:128, :])
    nc.gpsimd.tensor_mul(uk[1][32:64, :], mag[96:128, :], sin_a[96:128, :])

    # ---- scores (PE, one contraction-64 matmul per (h, kc)) + exp (ACT) ---
    exp_sb = [sb.tile([128, SC * 512], BF16, tag=f"exp{h}", name=f"exp{h}")
              for h in range(HPC)]
    for h in range(HPC):
        for kc in range(SC):
            sc_ps = ps.tile([128, 512], F32, tag="ps512")
            nc.tensor.matmul(sc_ps[:],
                             uk[h][:, kc * 128:(kc + 1) * 128],
                             uq[h][:, :],
                             start=True, stop=True)
            nc.scalar.activation(exp_sb[h][:, kc * 512:(kc + 1) * 512],
                                 sc_ps[:], AF.Exp)

    # ---- PV (PE): OutT_h rows 0:64 (d-major) + l row 64 -------------------
    out_sb = sb.tile([128, 512], BF16, tag="out_sb")
    rl_sb = [sb.tile([1, 512], F32, tag=f"rl{h}", name=f"rl{h}")
             for h in range(HPC)]
    for h in range(HPC):
        pv_ps = ps.tile([65, 512], F32, tag="ps512")
        for kc in range(SC):
            nc.tensor.matmul(pv_ps[:],
                             vp_sb[:, kc * 130 + 65 * h: kc * 130 + 65 * h + 65],
                             exp_sb[h][:, kc * 512:(kc + 1) * 512],
                             start=(kc == 0), stop=(kc == SC - 1))
        # custom-DVE ops give wrong results on HW when reading PSUM --
        # stage the l row through SBUF first
        l_sb = sb.tile([1, 512], F32, tag=f"l{h}", name=f"l{h}")
        nc.vector.tensor_copy(l_sb[:], pv_ps[64:65, :])
        nc.vector.reciprocal_approx_fast(rl_sb[h][:], l_sb[:])
        nc.vector.tensor_copy(out_sb[64 * h:64 * h + 64, :], pv_ps[0:64, :])

    # ---- 1/l transpose to partition-major (PE, one batched psum tile), ----
    # W_o partials, combine. rl_pm col 2*sc+h = 1/l for (chunk sc, head h).
    rlt_ps = ps.tile([128, 8], F32, tag="ps512")
    for sc in range(SC):
        for h in range(HPC):
            nc.tensor.matmul(rlt_ps[:, 2 * sc + h: 2 * sc + h + 1],
                             rl_sb[h][0:1, sc * 128:(sc + 1) * 128], one11,
                             start=True, stop=True)
    rl_pm = sb.tile([128, 8], F32, tag="rl_pm")
    nc.vector.tensor_copy(rl_pm[:], rlt_ps[:])

    for sc in range(SC):
        wo_ps = [ps.tile([128, 512], F32, tag="ps512", name=f"wo_ps{h}")
                 for h in range(HPC)]
        for h in range(HPC):
            nc.tensor.matmul(wo_ps[h][:],
                             out_sb[64 * h:64 * h + 64, sc * 128:(sc + 1) * 128],
                             wo_sb[64 * h:64 * h + 64, :],
                             start=True, stop=True)
        c1 = sb.tile([128, 512], F32, tag=f"c1_{sc}", name=f"c1_{sc}")
        nc.scalar.activation(c1[:], wo_ps[1][:], AF.Copy,
                             scale=rl_pm[:, 2 * sc + 1: 2 * sc + 2])
        fin = sb.tile([128, 512], BF16, tag=f"fin{sc}", name=f"fin{sc}")
        nc.vector.scalar_tensor_tensor(fin[:], wo_ps[0][:],
                                       rl_pm[:, 2 * sc: 2 * sc + 1], c1[:],
                                       op0=ALU.mult, op1=ALU.add)
        nc.sync.dma_start(out[sc * 128:(sc + 1) * 128, :], fin[:])


def build(reps=1):
    nc = bacc.Bacc("TRN2", target_bir_lowering=False, debug=False,
                   enable_asserts=False, num_devices=N_CORES)
    # Const [128,1] SBUF tensor for the Sqrt bias (only 0.0/1.0 pre-registered).
    for val in (1e-9,):
        t = nc.alloc_sbuf_tensor(f"const-f32-{val}", [128, 1], F32)
        nc.gpsimd.memset(t.ap(), val)
        nc.const_aps.aps[(F32, val)] = t.ap()
    nc.all_engine_barrier()
    aps = {
        "xq": nc.dram_tensor("xq", [D, S], F32, kind="ExternalInput").ap(),
        "xk": nc.dram_tensor("xk", [D, S], F32, kind="ExternalInput").ap(),
        "xv": nc.dram_tensor("xv", [D, S], BF16, kind="ExternalInput").ap(),
        "wq": nc.dram_tensor("wq", [128, 512], F32, kind="ExternalInput").ap(),
        "wk": nc.dram_tensor("wk", [128, 512], F32, kind="ExternalInput").ap(),
        "wv": nc.dram_tensor("wv", [128, 512], BF16, kind="ExternalInput").ap(),
        "wo": nc.dram_tensor("wo", [128, 512], BF16, kind="ExternalInput").ap(),
        "pf": nc.dram_tensor("pf", [128, 512], F32, kind="ExternalInput").ap(),
        "dl2": nc.dram_tensor("dl2", [128, 1], F32, kind="ExternalInput").ap(),
        "out": nc.dram_tensor("out", [S, D], BF16, kind="ExternalOutput").ap(),
    }
    with tile.TileContext(nc) as tc:
        with ExitStack() as ctx:
            sb = ctx.enter_context(tc.tile_pool(name="sb", bufs=1))
            ps = ctx.enter_context(tc.tile_pool(name="ps", bufs=6, space="PSUM"))
            for _ in range(reps):
                emit_body(nc, tc, ctx, aps, sb, ps)
    nc.compile()
    return nc


def make_in_maps(q, k, v, W_q, W_k, W_v, W_o, delta_params, bias_params):
    """Host-side shard prep: per-core input dicts. Core c = 4*b + hg."""
    bf16 = ml_dtypes.bfloat16
    freqs = 10000.0 ** (-np.arange(D_HALF, dtype=np.float32) * 2.0 / D_K)
    pos = np.arange(S, dtype=np.float32)
    posfreq = (freqs[:, None] * pos[None, :]).astype(np.float32)  # [32, 512]

    def sbuf_img(w_t, dtype):
        # [512, m] (contraction-major) -> SBUF image [128, 4*m]
        m = w_t.shape[1]
        return np.ascontiguousarray(
            w_t.reshape(JC, 128, m).transpose(1, 0, 2).reshape(128, JC * m)
        ).astype(dtype)

    xqs = [np.ascontiguousarray(np.asarray(q[b]).T, dtype=np.float32) for b in range(B)]
    xks = [np.ascontiguousarray(np.asarray(k[b]).T, dtype=np.float32) for b in range(B)]
    xvs = [np.ascontiguousarray(np.asarray(v[b]).T).astype(bf16) for b in range(B)]

    per_hg = []
    for hg in range(4):
        heads = [HPC * hg, HPC * hg + 1]
        perm = []
        for ri in range(2):
            for h in heads:
                perm.extend(range(D_K * h + 32 * ri, D_K * h + 32 * ri + 32))
        hslc = slice(128 * hg, 128 * hg + 128)
        pf = np.empty((128, 512), np.float32)
        dl2 = np.empty((128, 1), np.float32)
        for i, h in enumerate(heads):
            qr = slice(32 * i, 32 * i + 32)
            kr = slice(64 + 32 * i, 64 + 32 * i + 32)
            pf[qr] = (posfreq + np.asarray(bias_params)[h][:, None]) / (2 * np.pi)
            pf[kr] = posfreq / (2 * np.pi)
            dl2[qr, 0] = (2.0 / np.pi) * np.asarray(delta_params)[h]
            dl2[kr, 0] = (2.0 / np.pi) * np.asarray(delta_params)[h]
        per_hg.append({
            "wq": sbuf_img(np.asarray(W_q)[perm, :].T, np.float32),
            "wk": sbuf_img(np.asarray(W_k)[perm, :].T, np.float32),
            "wv": sbuf_img(np.asarray(W_v)[hslc, :].T, bf16),
            "wo": np.ascontiguousarray(np.asarray(W_o)[:, hslc].T).astype(bf16),
            "pf": pf,
            "dl2": dl2,
        })

    in_maps = []
    for c in range(N_CORES):
        b, hg = divmod(c, 4)
        m = dict(per_hg[hg])
        m["xq"] = xqs[b]
        m["xk"] = xks[b]
        m["xv"] = xvs[b]
        in_maps.append(m)
    return in_maps


_NC_CACHE = {}


def kernel(q, k, v, W_q, W_k, W_v, W_o, delta_params, bias_params):
    if "nc" not in _NC_CACHE:
        _NC_CACHE["nc"] = build(reps=1)
    nc = _NC_CACHE["nc"]
    in_maps = make_in_maps(q, k, v, W_q, W_k, W_v, W_o,
                           delta_params, bias_params)
    res = run_bass_kernel_spmd(nc, in_maps, core_ids=list(range(N_CORES)))
    outs = [res.results[c]["out"].astype(np.float32) for c in range(N_CORES)]
    final = np.empty((B, S, D), np.float32)
    for b in range(B):
        final[b] = outs[4 * b] + outs[4 * b + 1] + outs[4 * b + 2] + outs[4 * b + 3]
    return final
